# revision 5
# baseline (speedup 1.0000x reference)
"""Multi-head attention (B=8, N=1024, C=1024, H=16, D=64) with QK RMS-norm,
data-parallel across 8 NeuronCores (one batch element per core).

v2: bf16 matmul operands (fp32 PSUM accumulation), single continuous PE
stream with no phase barriers:
  - bias broadcasts + V(jh=0) tiles first (warms the PE HAM window)
  - per-pair q/k j-tiles + rms stats chains emitted as "filler" units that
    interleave into the attention pairs' st-loops
  - V(jh=1) tiles are filler for pairs 0-1
  - denominator normalize inlined per (pair, nh): reciprocal straight from
    the PSUM denominator row into the selector-broadcast staging tile
  - output projection immediately follows the last pair
"""

from collections import deque

import numpy as np

import concourse.bacc as bacc
import concourse.bass as bass
import concourse.tile as tile
from concourse import mybir
from concourse.bass_utils import run_bass_kernel_spmd

F32 = mybir.dt.float32
F8 = mybir.dt.float8e4
F32R = mybir.dt.float32r
BF16 = mybir.dt.bfloat16
AF = mybir.ActivationFunctionType
OP = mybir.AluOpType

B, N, C = 8, 1024, 1024
H, D = 16, 64
EPS = 1e-6
NCORES = 8
NT = N // 128      # token tiles
CT = C // 128      # channel tiles
NPAIR = H // 2     # head pairs


def _build():
    nc = bacc.Bacc(None, target_bir_lowering=False)

    xT_d = nc.dram_tensor("xT", [C, N], BF16, kind="ExternalInput")
    wqkvT_d = nc.dram_tensor("wqkvT", [C, 3 * C], BF16, kind="ExternalInput")
    wprojT_d = nc.dram_tensor("wprojT", [C, C], BF16, kind="ExternalInput")
    bqkv_d = nc.dram_tensor("bqkv", [3 * C], F32, kind="ExternalInput")
    bproj_d = nc.dram_tensor("bproj", [C], F32, kind="ExternalInput")
    # selector rows for K=2 broadcast matmuls at partition bases 0/32/64/96:
    # selq = 0/1 pattern, selk = same with qn*kn/sqrt(D) baked in.
    selq_d = nc.dram_tensor("selq", [8, 128], F32, kind="ExternalInput")
    selk_d = nc.dram_tensor("selk", [8, 128], F32, kind="ExternalInput")
    y_d = nc.dram_tensor("y", [N, C], BF16, kind="ExternalOutput")

    with tile.TileContext(nc) as tc:
        with (
            tc.tile_pool(name="lp", bufs=1) as lp,
            tc.tile_pool(name="wp", bufs=2) as wp,
            tc.tile_pool(name="psA", bufs=2, space="PSUM") as psA,
            tc.tile_pool(name="psS", bufs=2, space="PSUM") as psS,
            tc.tile_pool(name="psO", bufs=2, space="PSUM") as psO,
        ):
            # ---------------- long-lived SBUF ----------------
            xt = lp.tile([128, CT, N], BF16, tag="xt", name="xt")
            wvs = lp.tile([128, CT, C], BF16, tag="wvs", name="wvs")
            qk = [lp.tile([128, N], BF16, tag=f"qk{i}", name=f"qk{i}")
                  for i in range(2 * NPAIR)]
            vaug = [lp.tile([128, H, D + 1], BF16, tag=f"va{i}", name=f"va{i}")
                    for i in range(NT)]
            attnT = [lp.tile([128, N], BF16, tag=f"at{i}", name=f"at{i}")
                     for i in range(NPAIR)]
            wps = [lp.tile([128, CT, 512], BF16, tag=f"wps{i}", name=f"wps{i}")
                   for i in range(2)]
            bv_bc = lp.tile([128, C], F32, tag="bvbc", name="bvbc")
            bp_bc = lp.tile([128, C], F32, tag="bpbc", name="bpbc")
            ones2 = lp.tile([128, 2], BF16, tag="ones2", name="ones2")
            ones1 = lp.tile([1, 128], F32R, tag="ones1", name="ones1")
            selq = lp.tile([98, 128], F32R, tag="selq", name="selq")
            selk = lp.tile([98, 128], F32R, tag="selk", name="selk")
            rq_sb = [lp.tile([98, N], F32R, tag=f"rqs{g}", name=f"rqs{g}") for g in range(2)]
            rk_sb = [lp.tile([98, N], F32R, tag=f"rks{g}", name=f"rks{g}") for g in range(2)]
            dv_sb = [lp.tile([98, N], F32R, tag=f"dvs{g}", name=f"dvs{g}") for g in range(2)]
            eps_t = lp.tile([128, 1], F32, tag="epst", name="epst")
            bias_all = lp.tile([128, 2 * NPAIR], F32, tag="ball", name="ball")
            browv = lp.tile([1, C], F32R, tag="browv", name="browv")
            browp = lp.tile([1, C], F32R, tag="browp", name="browp")

            # ---------------- initial DMAs ----------------
            # sync (SP) queue carries ONLY small/latency-critical transfers;
            # all bulk goes on the scalar HWDGE queue (FIFO completion per
            # queue: a small DMA behind a bulk one waits for all of it)
            for j in range(4):
                nc.sync.dma_start(out=selq[32 * j:32 * j + 2, :],
                                  in_=selq_d[2 * j:2 * j + 2, :].bitcast(F32R))
                nc.sync.dma_start(out=selk[32 * j:32 * j + 2, :],
                                  in_=selk_d[2 * j:2 * j + 2, :].bitcast(F32R))
            nc.sync.dma_start(out=browv, in_=bqkv_d[2 * C:3 * C].unsqueeze(0).bitcast(F32R))
            nc.sync.dma_start(out=browp, in_=bproj_d[:].unsqueeze(0).bitcast(F32R))
            xsrc = xT_d[:, :].rearrange("(ct p) n -> p ct n", p=128)
            wvsrc = wqkvT_d[:, 2 * C:3 * C].rearrange("(ct p) j -> p ct j", p=128)
            # ALL bulk on the scalar HWDGE queue (the SP queue is the sem hub
            # and its DMAs head-of-line block behind semaphore waits): x
            # first (everything needs it), then pair-0 weights, then wvs
            nc.sync.dma_start(out=xt[:, 0:2, :], in_=xsrc[:, 0:2, :])
            nc.sync.dma_start(out=xt[:, 2:4, :], in_=xsrc[:, 2:4, :])
            nc.scalar.dma_start(out=xt[:, 4:6, :], in_=xsrc[:, 4:6, :])
            nc.scalar.dma_start(out=xt[:, 6:8, :], in_=xsrc[:, 6:8, :])
            nc.scalar.dma_start(out=bias_all,
                              in_=bqkv_d[0:2 * C].rearrange("(j p) -> p j", p=128))

            # ---------------- constants ----------------
            nc.vector.memset(ones2, 0.0)
            nc.vector.memset(ones2[0:64, 0:1], 1.0)
            nc.vector.memset(ones2[64:128, 1:2], 1.0)
            nc.vector.memset(ones1.bitcast(F32), 1.0)
            nc.vector.memset(eps_t, EPS)
            for nt in range(NT):
                nc.vector.memset(vaug[nt][:, :, D:D + 1], 1.0)

            # bias broadcasts via PE (also the first matmuls -> start HAM warm)
            for bi, (brow, btile) in enumerate(((browv, bv_bc), (browp, bp_bc))):
                for half in range(2):
                    hs = slice(half * 512, (half + 1) * 512)
                    bb = psA.tile([128, 512], F32, tag="mm512", name=f"bb{bi}{half}")
                    nc.tensor.matmul(out=bb, lhsT=ones1, rhs=brow[:, hs],
                                     start=True, stop=True)
                    nc.vector.tensor_copy(out=btile[:, hs], in_=bb)

            # remaining weight DMAs (wqkv j-tiles) are issued per-pair with
            # lead time; wproj during the late attention pairs.

            # ---------------- unit generators ----------------
            def v_unit(jh, nt):
                def emit():
                    vp = psA.tile([128, 512], F32, tag="mm512", name=f"vp{jh}{nt}")
                    for ct in range(CT):
                        nc.tensor.matmul(
                            out=vp,
                            lhsT=xt[:, ct, nt * 128:(nt + 1) * 128],
                            rhs=wvs[:, ct, jh * 512:(jh + 1) * 512],
                            start=(ct == 0), stop=(ct == CT - 1),
                        )
                    dst = vaug[nt][:, jh * 8:(jh + 1) * 8, 0:D]
                    src = vp[:, :].rearrange("p (h d) -> p h d", d=D)
                    bsrc = bv_bc[:, jh * 512:(jh + 1) * 512].rearrange(
                        "p (h d) -> p h d", d=D)
                    nc.vector.tensor_tensor(out=dst, in0=src, in1=bsrc, op=OP.add)
                return emit

            def load_dma(p):
                """Issue the weight DMAs for pair p's two j-tiles."""
                tiles = {}
                for jt, key in ((p, "q"), (NPAIR + p, "k")):
                    bias_c = bias_all[:, jt:jt + 1]
                    wts = wp.tile([128, CT, 128], BF16, tag="wts", bufs=4,
                                  name=f"wts{jt}")
                    nc.scalar.dma_start(
                        out=wts,
                        in_=wqkvT_d[:, jt * 128:(jt + 1) * 128]
                        .rearrange("(ct p) j -> p ct j", p=128),
                    )
                    tiles[key] = (bias_c, wts)
                return tiles

            def prep_units(p, tiles):
                """Units for pair p: q j-tile (jt=p), k j-tile (jt=8+p),
                stats chain, selector scale matmuls."""
                g, j = divmod(p, 4)
                state = {}

                def qknh(jt, key, nh):
                    def emit():
                        bias_c, wts = tiles[key]
                        nsl = slice(nh * 512, (nh + 1) * 512)
                        psum = psA.tile([128, 512], F32, tag="mm512",
                                        name=f"qkp{jt}{nh}")
                        for ct in range(CT):
                            nc.tensor.matmul(out=psum, lhsT=wts[:, ct, :],
                                             rhs=xt[:, ct, nsl],
                                             start=(ct == 0), stop=(ct == CT - 1))
                        nc.vector.tensor_scalar(
                            out=qk[jt][:, nsl], in0=psum,
                            scalar1=bias_c, scalar2=None, op0=OP.add)
                        if nh == 0:
                            state[key + "sq"] = wp.tile(
                                [128, N], BF16, tag="sq", bufs=2, name=f"sq{jt}")
                            state[key + "rp"] = wp.tile(
                                [2, N], F32, tag="rp", bufs=2, name=f"rp{jt}")
                        sq = state[key + "sq"]
                        nc.vector.tensor_mul(sq[:, nsl], qk[jt][:, nsl],
                                             qk[jt][:, nsl])
                    return emit

                def ssqnh(jt, key, nh):
                    # separate unit: the ssq matmul trails the DVE chain of
                    # its qknh unit, so keep other PE work between them
                    def emit():
                        nsl = slice(nh * 512, (nh + 1) * 512)
                        sq = state[key + "sq"]
                        rp = state[key + "rp"]
                        ssq = psA.tile([2, 512], F32, tag="mm512",
                                       name=f"ssq{jt}{nh}")
                        nc.tensor.matmul(out=ssq, lhsT=ones2, rhs=sq[:, nsl],
                                         start=True, stop=True)
                        nc.vector.tensor_copy(out=rp[:, nsl], in_=ssq)
                    return emit

                def stats():
                    def emit():
                        rpq, rpk = state["qrp"], state["krp"]
                        rrp = wp.tile([128, 32], F32, tag="rrp", bufs=2,
                                      name=f"rrp{p}")
                        nc.sync.dma_start(out=rrp[0:64, :], in_=rpq)
                        nc.sync.dma_start(out=rrp[64:128, :], in_=rpk)
                        # 1/sqrt(ssq/D + eps) via DVE Newton iteration (keeps
                        # the scalar engine exp-only: no act-table swaps)
                        mt = wp.tile([128, 32], F32, tag="mt", bufs=2,
                                     name=f"mt{p}")
                        yt = wp.tile([128, 32], F32, tag="yt", bufs=2,
                                     name=f"yt{p}")
                        ut = wp.tile([128, 32], F32, tag="ut", bufs=2,
                                     name=f"ut{p}")
                        nc.vector.tensor_scalar(out=mt, in0=rrp,
                                                scalar1=1.0 / D, scalar2=EPS,
                                                op0=OP.mult, op1=OP.add)
                        # minimax linear seed: 3 Newton iterations reach
                        # 2.6e-5 rel err over m in [0.32, 2.6]
                        nc.vector.tensor_scalar(out=yt, in0=mt,
                                                scalar1=-0.41, scalar2=1.57,
                                                op0=OP.mult, op1=OP.add)
                        nc.vector.tensor_scalar_max(out=yt, in0=yt, scalar1=0.15)
                        for _ in range(2):
                            nc.vector.tensor_mul(ut, yt, yt)
                            nc.vector.tensor_mul(ut, ut, mt)
                            nc.vector.tensor_scalar(out=ut, in0=ut,
                                                    scalar1=-0.5, scalar2=1.5,
                                                    op0=OP.mult, op1=OP.add)
                            nc.vector.tensor_mul(yt, yt, ut)
                        state["newton"] = (mt, yt, ut)
                    return emit

                def stats2():
                    def emit():
                        mt, yt, ut = state["newton"]
                        for _ in range(1):
                            nc.vector.tensor_mul(ut, yt, yt)
                            nc.vector.tensor_mul(ut, ut, mt)
                            nc.vector.tensor_scalar(out=ut, in0=ut,
                                                    scalar1=-0.5, scalar2=1.5,
                                                    op0=OP.mult, op1=OP.add)
                            nc.vector.tensor_mul(yt, yt, ut)
                        nc.sync.dma_start(out=rq_sb[g][32 * j:32 * j + 2, :],
                                          in_=yt[0:64, :].bitcast(F32R))
                        nc.sync.dma_start(out=rk_sb[g][32 * j:32 * j + 2, :],
                                          in_=yt[64:128, :].bitcast(F32R))
                    return emit

                def scale(jt, sel, r_sb):
                    def emit():
                        for nh in range(2):
                            nsl = slice(nh * 512, (nh + 1) * 512)
                            rqkp = psA.tile([128, 512], F32, tag="mm512",
                                            name=f"rqkp{jt}{nh}")
                            nc.tensor.matmul(out=rqkp,
                                             lhsT=sel[32 * j:32 * j + 2, :],
                                             rhs=r_sb[32 * j:32 * j + 2, nsl],
                                             start=True, stop=True,
                                             tile_position=(32 * j, 0))
                            nc.vector.tensor_mul(qk[jt][:, nsl], qk[jt][:, nsl],
                                                 rqkp)
                    return emit

                front = [
                    qknh(p, "q", 0), qknh(p, "q", 1), ssqnh(p, "q", 0),
                    qknh(NPAIR + p, "k", 0), ssqnh(p, "q", 1),
                    qknh(NPAIR + p, "k", 1), ssqnh(NPAIR + p, "k", 0),
                    ssqnh(NPAIR + p, "k", 1),
                    stats(), stats2(),
                ]
                back = [
                    scale(p, selq, rq_sb[g]), scale(NPAIR + p, selk, rk_sb[g]),
                ]
                return front, back

            filler = deque()  # entries: (pair_id, fn)
            pending_norms = []

            def pull(n=1):
                for _ in range(n):
                    if filler:
                        filler.popleft()[1]()

            def drain_through(pid):
                while filler and filler[0][0] <= pid:
                    filler.popleft()[1]()

            # ---------------- attention ----------------
            def pv(p, st, psb, o_e, o_o, stop=False):
                nc.tensor.matmul(
                    out=o_e, lhsT=vaug[st][:, 2 * p, :], rhs=psb[:, 0, :],
                    start=(st == 0), stop=stop,
                )
                nc.tensor.matmul(
                    out=o_o, lhsT=vaug[st][:, 2 * p + 1, :], rhs=psb[:, 1, :],
                    start=(st == 0), stop=stop,
                )

            def attn_pair(p):
                g, j = divmod(p, 4)
                ddr = wp.tile([128, 16], F32, tag="ddr", bufs=2, name=f"ddr{p}")
                for nh in range(2):
                    nsl = slice(nh * 512, (nh + 1) * 512)
                    o_e = psO.tile([D + 1, 512], F32, tag="ov", name=f"oe{p}{nh}")
                    o_o = psO.tile([D + 1, 512], F32, tag="ov", name=f"oo{p}{nh}")
                    psbs = []
                    for st in range(NT):
                        sps = psS.tile([128, 2, 512], F32, tag="sps",
                                       name=f"sps{p}{nh}{st}")
                        nc.tensor.matmul(
                            out=sps[:, 0, :],
                            lhsT=qk[NPAIR + p][0:64, st * 128:(st + 1) * 128],
                            rhs=qk[p][0:64, nsl],
                            start=True, stop=True, tile_position=(0, 0),
                        )
                        nc.tensor.matmul(
                            out=sps[:, 1, :],
                            lhsT=qk[NPAIR + p][64:128, st * 128:(st + 1) * 128],
                            rhs=qk[p][64:128, nsl],
                            start=True, stop=True, tile_position=(64, 0),
                        )
                        psb = wp.tile([128, 2, 512], BF16, tag="psb", bufs=3,
                                      name=f"psb{p}{nh}{st}")
                        nc.scalar.activation(out=psb, in_=sps, func=AF.Exp)
                        psbs.append(psb)
                        # filler BEFORE the dependent PV: the PE queue is
                        # strict FIFO, so the filler's matmuls cover the
                        # exp latency (and the o-ring handoff at st==1)
                        pull(1)
                        if nh == 0 and st == 6 and pending_norms:
                            for fn in pending_norms:
                                fn()
                            pending_norms.clear()
                        if st > 0:
                            pv(p, st - 1, psbs[st - 1], o_e, o_o)
                    pv(p, NT - 1, psbs[NT - 1], o_e, o_o, stop=True)
                    # denominator rows -> SBUF -> packed [128,16] staging
                    dstg_e = wp.tile([1, 512], F32, tag="dstge", bufs=2,
                                     name=f"dstge{p}{nh}")
                    dstg_o = wp.tile([1, 512], F32, tag="dstgo", bufs=2,
                                     name=f"dstgo{p}{nh}")
                    nc.vector.tensor_copy(out=dstg_e, in_=o_e[D:D + 1, :])
                    nc.vector.tensor_copy(out=dstg_o, in_=o_o[D:D + 1, :])
                    nc.sync.dma_start(out=ddr[64 * nh:64 * nh + 32, :],
                                      in_=dstg_e)
                    nc.sync.dma_start(out=ddr[64 * nh + 32:64 * nh + 64, :],
                                      in_=dstg_o)
                    # attnT evictions on the scalar engine: frees the o PSUM
                    # ring without queueing behind the DVE filler backlog
                    nc.scalar.activation(out=attnT[p][0:64, nsl],
                                         in_=o_e[0:D, :], func=AF.Copy)
                    nc.scalar.activation(out=attnT[p][64:128, nsl],
                                         in_=o_o[0:D, :], func=AF.Copy)
                # one packed reciprocal per pair, unpack into selector rows
                nc.vector.reciprocal(out=ddr, in_=ddr)
                for nh in range(2):
                    nsl = slice(nh * 512, (nh + 1) * 512)
                    for r in range(2):
                        qsl = slice(64 * nh + 32 * r, 64 * nh + 32 * r + 32)
                        nc.sync.dma_start(
                            out=dv_sb[g][32 * j + r:32 * j + r + 1, nsl],
                            in_=ddr[qsl, :].bitcast(F32R))

                    def norm_unit(nh=nh, nsl=nsl):
                        dbp = psA.tile([128, 512], F32, tag="mm512",
                                       name=f"dbp{p}{nh}")
                        nc.tensor.matmul(out=dbp,
                                         lhsT=selq[32 * j:32 * j + 2, :],
                                         rhs=dv_sb[g][32 * j:32 * j + 2, nsl],
                                         start=True, stop=True,
                                         tile_position=(32 * j, 0))
                        nc.vector.tensor_mul(attnT[p][:, nsl], attnT[p][:, nsl],
                                             dbp)
                    pending_norms.append(norm_unit)

            # ---------------- emission schedule ----------------
            # pre-attention: V(jh=0) interleaved with prep(0)
            tiles0 = load_dma(0)
            nc.scalar.dma_start(out=wvs[:, :, 0:512], in_=wvsrc[:, :, 0:512])
            nc.scalar.dma_start(out=wvs[:, :, 512:1024], in_=wvsrc[:, :, 512:1024])
            f0, b0 = prep_units(0, tiles0)
            v0 = [v_unit(0, nt) for nt in range(NT)]
            # qk units first (need only x + wts), V after (needs wvs), scales
            # last (stats chain latency covered by the V units)
            for u in f0 + v0 + b0:
                u()

            spacer = lambda: None  # noqa: E731
            v1 = [v_unit(1, nt) for nt in range(NT)]
            v1_share = {1: v1[0:3], 2: v1[3:6], 3: v1[6:8]}
            for p in range(NPAIR):
                if p + 1 < NPAIR:
                    tiles = load_dma(p + 1)
                    front, back = prep_units(p + 1, tiles)
                    units = (front + v1_share.get(p, [spacer, spacer])
                             + [spacer, spacer] + back)
                    filler.extend((p + 1, u) for u in units)
                drain_through(p)  # pair p's own prep MUST precede its attention
                if p == 3:
                    wpsrc = wprojT_d[:, :].rearrange("(ct p) j -> p ct j", p=128)
                    for ch in range(2):
                        for q4 in range(4):
                            j0 = ch * 512 + q4 * 128
                            nc.scalar.dma_start(
                                out=wps[ch][:, :, q4 * 128:q4 * 128 + 128],
                                in_=wpsrc[:, :, j0:j0 + 128],
                            )
                attn_pair(p)
            drain_through(NPAIR)

            # ---------------- output projection ----------------
            # first two groups split their accumulation around pair-7's norm
            # units so those selector matmuls' reciprocal/DMA chain is covered
            def proj_group(nt, ch, ct_hi, pool=None, tag="mm512"):
                yp = (pool or psA).tile([128, 512], F32, tag=tag, name=f"yp{nt}{ch}")
                for ct in range(ct_hi):
                    nc.tensor.matmul(
                        out=yp,
                        lhsT=attnT[ct][:, nt * 128:(nt + 1) * 128],
                        rhs=wps[ch][:, ct, :],
                        start=(ct == 0), stop=(ct == CT - 1),
                    )
                return yp

            def proj_finish(nt, ch, yp, ct_lo):
                for ct in range(ct_lo, CT):
                    nc.tensor.matmul(
                        out=yp,
                        lhsT=attnT[ct][:, nt * 128:(nt + 1) * 128],
                        rhs=wps[ch][:, ct, :],
                        start=(ct == 0), stop=(ct == CT - 1),
                    )
                ysb = wp.tile([128, 512], BF16, tag="ysb", bufs=3,
                              name=f"ysb{nt}{ch}")
                nc.vector.tensor_tensor(
                    out=ysb, in0=yp,
                    in1=bp_bc[:, ch * 512:(ch + 1) * 512], op=OP.add)
                eng = nc.sync if (nt + ch) % 2 else nc.scalar
                eng.dma_start(
                    out=y_d[nt * 128:(nt + 1) * 128, ch * 512:(ch + 1) * 512],
                    in_=ysb,
                )

            # borrow the (now idle) sps ring for these two so the norm
            # units' dbp matmuls can still allocate from the mm512 ring
            yp00 = proj_group(0, 0, CT - 1, pool=psS, tag="sps")
            yp01 = proj_group(0, 1, CT - 1, pool=psS, tag="sps")
            for fn in pending_norms:
                fn()
            pending_norms.clear()
            proj_finish(0, 0, yp00, CT - 1)
            proj_finish(0, 1, yp01, CT - 1)
            for nt in range(NT):
                for ch in range(2):
                    if nt == 0:
                        continue
                    yp = proj_group(nt, ch, CT)
                    proj_finish(nt, ch, yp, CT)
    nc.compile()
    return nc


_NC = None


def _get_nc():
    global _NC
    if _NC is None:
        _NC = _build()
    return _NC


def make_in_maps(x, w_qkv, b_qkv, qn_w, kn_w, w_proj, b_proj):
    import ml_dtypes
    bf16 = ml_dtypes.bfloat16

    x = np.asarray(x, dtype=np.float32)
    xT = np.ascontiguousarray(np.transpose(x, (0, 2, 1))).astype(bf16)  # [B, C, N]
    wqkvT = np.ascontiguousarray(np.asarray(w_qkv, np.float32).T).astype(bf16)
    wprojT = np.ascontiguousarray(np.asarray(w_proj, np.float32).T).astype(bf16)
    scale = np.float32(1.0) / np.sqrt(np.float32(D)).astype(np.float32)
    qnkn = (np.asarray(qn_w, np.float32) * np.asarray(kn_w, np.float32) * scale)
    selq = np.zeros((8, 128), np.float32)
    selk = np.zeros((8, 128), np.float32)
    for g in range(4):
        selq[2 * g, 0:64] = 1.0
        selq[2 * g + 1, 64:128] = 1.0
        selk[2 * g, 0:64] = qnkn
        selk[2 * g + 1, 64:128] = qnkn
    return [
        {
            "xT": xT[b],
            "wqkvT": wqkvT,
            "wprojT": wprojT,
            "bqkv": np.asarray(b_qkv, np.float32),
            "bproj": np.asarray(b_proj, np.float32),
            "selq": selq,
            "selk": selk,
        }
        for b in range(B)
    ]


def kernel(x, w_qkv, b_qkv, qn_w, kn_w, w_proj, b_proj, **_ignored):
    nc = _get_nc()
    in_maps = make_in_maps(x, w_qkv, b_qkv, qn_w, kn_w, w_proj, b_proj)
    res = run_bass_kernel_spmd(nc, in_maps, core_ids=list(range(NCORES)))
    return np.stack([res.results[b]["y"] for b in range(B)]).astype(np.float32)
